# revision 22
# baseline (speedup 1.0000x reference)
"""Contrastive projection head loss on 8 Trainium2 NeuronCores.

Reference computation (B=8192, E=1024, P=512):
    z_codon = relu(x[:, :E]) @ w + b          # [B, P]
    z_amino = relu(x[:, E:]) @ w + b          # [B, P]
    z  = concat([z_codon, z_amino], axis=1)   # [B, 2P]
    zn = z / max(||z||, 1e-8)
    s  = (zn @ zn.T);  s[i,i] = -9e15;  s /= 0.1
    nll_i = -s[i, (i - B/2) % B] + logsumexp(s[i, :])
    out = mean(nll)

Distribution: data-parallel over B (1024 rows/core). Each core projects
(bf16 GEMM, PE transposes via bf16 identity) and normalizes its rows (kept
feature-major as zn^T, stored as 16*zn in fp8e4 so values sit in e4m3's
normal range; the 1/256 is folded into the exp/temperature scale) and
all-gathers zn^T in two column chunks, pipelined with phase 1 by
row-halves. The similarity GEMM runs in fp8 DoubleRow mode (two
128-feature blocks per matmul, 2x bf16 rate). Column-sum exchanges use one
ReduceScatter per chunk so the first overlaps the second chunk's compute.

The similarity matrix is symmetric, so each core computes only the block
column range d = 0..4 (its own rows against cores c..c+4 mod 8), halving
the GEMM. Blocks d=1..3 additionally produce column sums of exp(s/T) (one
PE ones-matmul per tile) which are routed to the owning cores with a
ReduceScatter; block d=4 is computed by both endpoints (row sums only), so
every row's logsumexp denominator is covered exactly once. Remote operands
are addressed with partition-id-derived dynamic DMA offsets, keeping the
SPMD program identical on all cores. The self-similarity term is removed
by subtracting its exp (block d=0, compile-time positions); the
positive-pair logit is the block diagonal of the d=4 block.

Returns per-core partial sums [1, 8]; host sums and divides by B.
"""
import numpy as np

from concourse import bass, mybir, tile, bacc
from concourse.bass_utils import run_bass_kernel_spmd
from concourse.masks import make_identity

N_CORES = 8
B = 8192
E = 1024          # embedding size (per half)
P = 512           # projection size
D = 2 * P         # z feature dim = 1024
R = B // N_CORES  # rows per core = 1024
KT = D // 128     # feature sub-tiles = 8
MT = R // 128     # row sub-tiles per core = 8
INV_T = 10.0      # 1 / temperature
EPS = 1e-8

F32 = mybir.dt.float32
F32R = mybir.dt.float32r
BF16 = mybir.dt.bfloat16
FP8 = mybir.dt.float8e4
GDT = FP8         # dtype of gathered zn^T / similarity-GEMM operands
DR = mybir.MatmulPerfMode.DoubleRow
SCL = 16.0        # zn is stored as SCL*zn in fp8 to stay in normal range
INV_TS = 10.0 / (SCL * SCL)   # folds 1/SCL^2 into the exp/temperature scale
AF = mybir.ActivationFunctionType
ALU = mybir.AluOpType

NSLOT = 10        # rowsum slots: 2 local (d=0) + 8 remote (d=1..4, k=0..1)

_cached = {}


def _build(no_collective=False):
    nc = bacc.Bacc("TRN2", target_bir_lowering=False, debug=False,
                   enable_asserts=False, num_devices=N_CORES)
    x_in = nc.dram_tensor("xs", [R, 2 * E], F32, kind="ExternalInput").ap()
    w_in = nc.dram_tensor("w", [E, P], F32, kind="ExternalInput").ap()
    b_in = nc.dram_tensor("b", [P], F32, kind="ExternalInput").ap()
    out = nc.dram_tensor("out", [1, MT], F32, kind="ExternalOutput").ap()

    with tile.TileContext(nc) as tc:
        with tc.tile_pool(name="const", bufs=1) as const, \
             tc.tile_pool(name="big", bufs=2) as big, \
             tc.tile_pool(name="small", bufs=1) as small, \
             tc.tile_pool(name="dram", bufs=1, space="DRAM") as dram:

            ident = const.tile([128, 128], F32)
            make_identity(nc, ident[:])
            ident_b = const.tile([128, 128], BF16)
            nc.vector.tensor_copy(ident_b[:], ident[:])
            ones_f = const.tile([128, 1], F32)
            nc.vector.memset(ones_f[:], 1.0)
            ones_b = const.tile([128, 1], BF16)
            nc.vector.tensor_copy(ones_b[:], ones_f[:])
            b2 = const.tile([128, P // 128], F32)
            nc.scalar.dma_start(b2[:], b_in.rearrange("(mt p) -> p mt", p=128))
            rn_bc = const.tile([128, R], F32)

            # w as [128, KT(=E/128), P] fp8, stored as SCL*w so values sit
            # in e4m3's normal range; 1/SCL is folded into the bias-add.
            # Issued on the scalar queue so the x row loads go first on sync.
            w_r = const.tile([128, E // 128, P], FP8)
            with tc.tile_pool(name="wst", bufs=1) as wst:
                wstage = wst.tile([128, E // 128, P], F32, tag="wstage")
                nc.scalar.dma_start(wstage[:],
                                    w_in.rearrange("(kt p) q -> p kt q", p=128))
                nc.vector.tensor_scalar_mul(w_r[:], wstage[:], SCL)

            # z^T feature-major, f32r; znT is the normalized bf16 copy
            zT = big.tile([128, KT, R], F32R, tag="z")
            znT = big.tile([128, KT, R], GDT, tag="z")
            ag_in = [dram.tile([D, 512], GDT, name=f"ag_in{k}")
                     for k in range(2)]
            ag_out = [dram.tile([N_CORES * D, 512], GDT, name=f"ag_out{k}",
                                addr_space="Local" if no_collective else "Shared")
                      for k in range(2)]
            rn_dram = dram.tile([R], F32)
            rs_in = [dram.tile([N_CORES, 512], F32, name=f"rs_in{k}")
                     for k in range(2)]
            rs_out = [dram.tile([512], F32, name=f"rs_out{k}")
                      for k in range(2)]

            # ---- phase 1, pipelined over row-halves jh ----
            with tc.tile_pool(name="xrow", bufs=2) as xrowp, \
                 tc.tile_pool(name="xr16", bufs=2) as xr16p, \
                 tc.tile_pool(name="xTp", bufs=2) as xTp, \
                 tc.tile_pool(name="sqp", bufs=2) as sqp, \
                 tc.tile_pool(name="ps1", bufs=2, space="PSUM") as ps1:
                for jh in range(2):
                    # relu+cast rows to bf16, then PE-transpose (bf16 1cyc/row)
                    xT = xTp.tile([128, 2 * KT, 512], FP8, tag="xT",
                                  name=f"xT{jh}")
                    for r in range(4):
                        rg = jh * 4 + r
                        xrow = xrowp.tile([128, 2 * E], F32, tag="xrow")
                        nc.sync.dma_start(xrow[:],
                                          x_in[rg * 128:(rg + 1) * 128, :])
                        xrow16 = xr16p.tile([128, 2 * E], BF16, tag="xr16")
                        nc.vector.tensor_scalar_max(xrow16[:], xrow[:], 0.0)
                        for cg in range(2 * E // 512):
                            pt = ps1.tile([128, 4, 128], BF16, tag="tp",
                                          bufs=3)
                            for q in range(4):
                                ct = cg * 4 + q
                                nc.tensor.transpose(
                                    pt[:, q, :],
                                    xrow16[:, ct * 128:(ct + 1) * 128],
                                    ident_b[:])
                            nc.vector.tensor_copy(
                                xT[:, cg * 4:(cg + 1) * 4,
                                   r * 128:(r + 1) * 128],
                                pt[:])
                    # project this half, interleaving the row-norm
                    # accumulation so pn completes right after the last tile
                    pn = ps1.tile([1, 512], F32, tag="pn", bufs=2,
                                  name=f"pn{jh}")
                    for h in range(2):
                        for m4 in range(P // 128):
                            ktile = h * 4 + m4
                            pz = ps1.tile([128, 512], F32, tag="pz", bufs=2)
                            for t in range(E // 256):
                                nc.tensor.matmul(
                                    pz[:],
                                    w_r[:, 2 * t:2 * t + 2,
                                        m4 * 128:(m4 + 1) * 128],
                                    xT[:, h * KT + 2 * t:h * KT + 2 * t + 2,
                                       :],
                                    start=(t == 0), stop=(t == E // 256 - 1),
                                    perf_mode=DR)
                            zsl = zT[:, ktile, jh * 512:(jh + 1) * 512]
                            nc.vector.tensor_scalar(
                                zsl, pz[:], 1.0 / SCL, b2[:, m4:m4 + 1],
                                ALU.mult, ALU.add)
                            sq = sqp.tile([128, 512], BF16, tag="sq")
                            nc.vector.tensor_tensor(sq[:], zsl, zsl, ALU.mult)
                            nc.tensor.matmul(pn[:], ones_b[:], sq[:],
                                             start=(ktile == 0),
                                             stop=(ktile == KT - 1))
                    nrm = small.tile([1, 512], F32, tag="nrm", name=f"nrm{jh}")
                    # sqrt(pn)/SCL, so the reciprocal below yields SCL/||z||
                    nc.scalar.activation(nrm[:], pn[:], AF.Sqrt,
                                         scale=1.0 / (SCL * SCL))
                    nc.scalar.dma_start(rn_dram[None, jh * 512:(jh + 1) * 512],
                                        nrm[:])
                    nrm_bc = small.tile([128, 512], F32, tag="nrmbc",
                                        name=f"nrmbc{jh}")
                    nc.scalar.dma_start(
                        nrm_bc[:],
                        rn_dram[None, jh * 512:(jh + 1) * 512]
                        .to_broadcast([128, 512]))
                    # reciprocal on all 128 partitions (1-lane recip is slow)
                    nc.vector.tensor_scalar_max(nrm_bc[:], nrm_bc[:], EPS)
                    nc.vector.reciprocal(
                        rn_bc[:, jh * 512:(jh + 1) * 512], nrm_bc[:])
                    # normalize and ship this half
                    for kt in range(KT):
                        nc.vector.tensor_tensor(
                            znT[:, kt, jh * 512:(jh + 1) * 512],
                            zT[:, kt, jh * 512:(jh + 1) * 512],
                            rn_bc[:, jh * 512:(jh + 1) * 512], ALU.mult)
                    nc.scalar.dma_start(
                        ag_in[jh].rearrange("(kt p) j -> p kt j", p=128),
                        znT[:, :, jh * 512:(jh + 1) * 512])
                    if no_collective:
                        for c in range(N_CORES):
                            nc.sync.dma_start(
                                ag_out[jh][c * D:(c + 1) * D, :], ag_in[jh][:])
                    else:
                        nc.gpsimd.collective_compute(
                            "AllGather", ALU.bypass,
                            replica_groups=[list(range(N_CORES))],
                            ins=[ag_in[jh][:]], outs=[ag_out[jh][:]])

            # ---- phase 2: symmetric blockwise cos-sim ----
            rowsum = const.tile([128, MT, NSLOT], F32)
            pos_acc = const.tile([128, MT], F32)
            corr_acc = const.tile([128, MT], F32)

            pid = nc.sync.partition_id()

            def gemm_tile(pg, rhs_ap, m):
                # fp8 DoubleRow: each matmul contracts two 128-feature blocks
                for t in range(KT // 2):
                    nc.tensor.matmul(pg[:],
                                     znT[:, 2 * t:2 * t + 2,
                                         m * 128:(m + 1) * 128],
                                     rhs_ap[:, 2 * t:2 * t + 2, :],
                                     start=(t == 0), stop=(t == KT // 2 - 1),
                                     perf_mode=DR)

            with tc.tile_pool(name="rhs", bufs=2) as rhsp, \
                 tc.tile_pool(name="junk", bufs=3) as junkp, \
                 tc.tile_pool(name="dtmp", bufs=4) as dtmpp, \
                 tc.tile_pool(name="ps2", bufs=1, space="PSUM") as ps2:

                # zero the ReduceScatter inputs (unwritten slots must be 0)
                zb = small.tile([N_CORES, 512], F32)
                nc.vector.memset(zb[:], 0.0)
                for k in range(2):
                    nc.sync.dma_start(rs_in[k][:], zb[:])

                # local block d=0 first — overlaps the collectives
                for nb in range(2):
                    for m in range(MT):
                        pg = ps2.tile([128, 512], F32, tag="pg", bufs=5,
                                      name=f"pgl{nb}_{m}")
                        gemm_tile(pg, znT[:, :, nb * 512:(nb + 1) * 512], m)
                        junk = junkp.tile([128, 512], BF16, tag="junk")
                        nc.scalar.activation(
                            junk[:], pg[:], AF.Exp, scale=INV_TS,
                            accum_out=rowsum[:, m, nb:nb + 1])
                        if nb == m // 4:
                            # self-similarity at compile-time position
                            off = (m % 4) * 128
                            jd = junkp.tile([128, 128], F32, tag="jd")
                            nc.vector.tensor_tensor(
                                jd[:], pg[:, off:off + 128], ident[:],
                                ALU.mult)
                            d = dtmpp.tile([128, 1], F32, tag="d")
                            nc.vector.reduce_sum(d[:], jd[:],
                                                 axis=mybir.AxisListType.X)
                            nc.scalar.activation(
                                corr_acc[:, m:m + 1], d[:], AF.Exp,
                                scale=INV_TS)

                # remote blocks d = 1..4, per gathered chunk k
                for k in range(2):
                    for dd in range(1, 5):
                        slot = 2 + (dd - 1) * 2 + k
                        row0 = ((pid + dd) % N_CORES) * D
                        rhs = rhsp.tile([128, KT, 512], GDT, tag="rhs")
                        src = ag_out[k][bass.ds(row0, D), :].rearrange(
                            "(kt p) j -> p kt j", p=128)
                        nc.sync.dma_start(rhs[:], src)
                        cs = None
                        if dd < 4:
                            cs = ps2.tile([1, 512], F32, tag="cs", bufs=2,
                                          name=f"cs{k}_{dd}")
                        for m in range(MT):
                            pg = ps2.tile([128, 512], F32, tag="pg", bufs=5,
                                          name=f"pg{k}_{dd}_{m}")
                            gemm_tile(pg, rhs, m)
                            junk = junkp.tile([128, 512], BF16, tag="junk")
                            nc.scalar.activation(
                                junk[:], pg[:], AF.Exp, scale=INV_TS,
                                accum_out=rowsum[:, m, slot:slot + 1])
                            if dd < 4:
                                nc.tensor.matmul(cs[:], ones_b[:], junk[:],
                                                 start=(m == 0),
                                                 stop=(m == MT - 1))
                            if dd == 4 and k == m // 4:
                                # positive-pair logit on the block diagonal
                                off = (m % 4) * 128
                                jd = junkp.tile([128, 128], F32, tag="jd")
                                nc.vector.tensor_tensor(
                                    jd[:], pg[:, off:off + 128], ident[:],
                                    ALU.mult)
                                dpos = dtmpp.tile([128, 1], F32, tag="dp")
                                nc.vector.reduce_sum(
                                    dpos[:], jd[:], axis=mybir.AxisListType.X)
                                nc.vector.tensor_scalar_mul(
                                    pos_acc[:, m:m + 1], dpos[:], INV_TS)
                        if dd < 4:
                            # ship this block's column sums to core (c+dd)
                            css = dtmpp.tile([1, 512], F32, tag="css",
                                             name=f"css{k}_{dd}")
                            nc.vector.tensor_copy(css[:], cs[:])
                            nc.sync.dma_start(
                                rs_in[k][bass.ds((pid + dd) % N_CORES, 1), :],
                                css[:])
                    # per-chunk ReduceScatter: k=0's overlaps k=1 compute
                    if dd == 4:
                        if no_collective:
                            nc.sync.dma_start(rs_out[k][None, :],
                                              rs_in[k][0:1, :])
                        else:
                            nc.gpsimd.collective_compute(
                                "ReduceScatter", ALU.add,
                                replica_groups=[list(range(N_CORES))],
                                ins=[rs_in[k][:]], outs=[rs_out[k][:]])

                # ---- finale: lse, nll, partial sum (batched over m) ----
                rs = small.tile([128, MT], F32)
                nc.vector.reduce_sum(rs[:], rowsum[:],
                                     axis=mybir.AxisListType.X)
                rcv = small.tile([128, MT], F32)
                for k in range(2):
                    nc.sync.dma_start(
                        rcv[:, k * 4:(k + 1) * 4],
                        rs_out[k].rearrange("(m p) -> p m", p=128))
                nc.vector.tensor_tensor(rs[:], rs[:], rcv[:], ALU.add)
                nc.vector.tensor_tensor(rs[:], rs[:], corr_acc[:], ALU.subtract)
                lse = small.tile([128, MT], F32)
                nc.scalar.activation(lse[:], rs[:], AF.Ln)
                nll = small.tile([128, MT], F32)
                nc.vector.tensor_tensor(nll[:], lse[:], pos_acc[:], ALU.subtract)
                pf = ps2.tile([1, MT], F32, tag="pf", bufs=1)
                nc.tensor.matmul(pf[:], ones_f[:], nll[:], start=True, stop=True)
                fs = small.tile([1, MT], F32)
                nc.vector.tensor_copy(fs[:], pf[:])
                nc.sync.dma_start(out[:], fs[:])

    nc.compile()
    return nc


def kernel(x, w, b):
    if "nc" not in _cached:
        _cached["nc"] = _build()
    nc = _cached["nc"]
    x = np.ascontiguousarray(np.asarray(x, dtype=np.float32))
    w = np.ascontiguousarray(np.asarray(w, dtype=np.float32))
    b = np.ascontiguousarray(np.asarray(b, dtype=np.float32))
    in_maps = [{
        "xs": np.ascontiguousarray(x[c * R:(c + 1) * R]),
        "w": w, "b": b,
    } for c in range(N_CORES)]
    res = run_bass_kernel_spmd(nc, in_maps, list(range(N_CORES)))
    total = 0.0
    for c in range(N_CORES):
        total += float(res.results[c]["out"].astype(np.float64).sum())
    return np.float32(total / B)



# revision 23
# speedup vs baseline: 1.0115x; 1.0115x over previous
"""Contrastive projection head loss on 8 Trainium2 NeuronCores.

Reference computation (B=8192, E=1024, P=512):
    z_codon = relu(x[:, :E]) @ w + b          # [B, P]
    z_amino = relu(x[:, E:]) @ w + b          # [B, P]
    z  = concat([z_codon, z_amino], axis=1)   # [B, 2P]
    zn = z / max(||z||, 1e-8)
    s  = (zn @ zn.T);  s[i,i] = -9e15;  s /= 0.1
    nll_i = -s[i, (i - B/2) % B] + logsumexp(s[i, :])
    out = mean(nll)

Distribution: data-parallel over B (1024 rows/core). Each core projects
(bf16 GEMM, PE transposes via bf16 identity) and normalizes its rows (kept
feature-major as zn^T, stored as 16*zn in fp8e4 so values sit in e4m3's
normal range; the 1/256 is folded into the exp/temperature scale) and
all-gathers zn^T in two column chunks, pipelined with phase 1 by
row-halves. The similarity GEMM runs in fp8 DoubleRow mode (two
128-feature blocks per matmul, 2x bf16 rate). Column-sum exchanges use one
ReduceScatter per chunk so the first overlaps the second chunk's compute.

The similarity matrix is symmetric, so each core computes only the block
column range d = 0..4 (its own rows against cores c..c+4 mod 8), halving
the GEMM. Blocks d=1..3 additionally produce column sums of exp(s/T) (one
PE ones-matmul per tile) which are routed to the owning cores with a
ReduceScatter; block d=4 is computed by both endpoints (row sums only), so
every row's logsumexp denominator is covered exactly once. Remote operands
are addressed with partition-id-derived dynamic DMA offsets, keeping the
SPMD program identical on all cores. The self-similarity term is removed
by subtracting its exp (block d=0, compile-time positions); the
positive-pair logit is the block diagonal of the d=4 block.

Returns per-core partial sums [1, 8]; host sums and divides by B.
"""
import numpy as np

from concourse import bass, mybir, tile, bacc
from concourse.bass_utils import run_bass_kernel_spmd
from concourse.masks import make_identity

N_CORES = 8
B = 8192
E = 1024          # embedding size (per half)
P = 512           # projection size
D = 2 * P         # z feature dim = 1024
R = B // N_CORES  # rows per core = 1024
KT = D // 128     # feature sub-tiles = 8
MT = R // 128     # row sub-tiles per core = 8
INV_T = 10.0      # 1 / temperature
EPS = 1e-8

F32 = mybir.dt.float32
F32R = mybir.dt.float32r
BF16 = mybir.dt.bfloat16
FP8 = mybir.dt.float8e4
GDT = FP8         # dtype of gathered zn^T / similarity-GEMM operands
DR = mybir.MatmulPerfMode.DoubleRow
SCL = 16.0        # zn is stored as SCL*zn in fp8 to stay in normal range
INV_TS = 10.0 / (SCL * SCL)   # folds 1/SCL^2 into the exp/temperature scale
AF = mybir.ActivationFunctionType
ALU = mybir.AluOpType

NSLOT = 10        # rowsum slots: 2 local (d=0) + 8 remote (d=1..4, k=0..1)

_cached = {}


def _build(no_collective=False):
    nc = bacc.Bacc("TRN2", target_bir_lowering=False, debug=False,
                   enable_asserts=False, num_devices=N_CORES)
    x_in = nc.dram_tensor("xs", [R, 2 * E], F32, kind="ExternalInput").ap()
    w_in = nc.dram_tensor("w", [E, P], F32, kind="ExternalInput").ap()
    b_in = nc.dram_tensor("b", [P], F32, kind="ExternalInput").ap()
    out = nc.dram_tensor("out", [1, MT], F32, kind="ExternalOutput").ap()

    with tile.TileContext(nc) as tc:
        with tc.tile_pool(name="const", bufs=1) as const, \
             tc.tile_pool(name="big", bufs=2) as big, \
             tc.tile_pool(name="small", bufs=1) as small, \
             tc.tile_pool(name="dram", bufs=1, space="DRAM") as dram:

            ident = const.tile([128, 128], F32)
            make_identity(nc, ident[:])
            ident_b = const.tile([128, 128], BF16)
            nc.vector.tensor_copy(ident_b[:], ident[:])
            ones_f = const.tile([128, 1], F32)
            nc.vector.memset(ones_f[:], 1.0)
            ones_b = const.tile([128, 1], BF16)
            nc.vector.tensor_copy(ones_b[:], ones_f[:])
            b2 = const.tile([128, P // 128], F32)
            nc.scalar.dma_start(b2[:], b_in.rearrange("(mt p) -> p mt", p=128))
            rn_bc = const.tile([128, R], F32)

            # w as [128, KT(=E/128), P] bf16 — staged in a scoped pool.
            # Issued on the scalar queue so the x row loads go first on sync.
            w_r = const.tile([128, E // 128, P], BF16)
            with tc.tile_pool(name="wst", bufs=1) as wst:
                wstage = wst.tile([128, E // 128, P], F32, tag="wstage")
                nc.scalar.dma_start(wstage[:],
                                    w_in.rearrange("(kt p) q -> p kt q", p=128))
                nc.vector.tensor_copy(w_r[:], wstage[:])

            # z^T feature-major, f32r; znT is the normalized bf16 copy
            zT = big.tile([128, KT, R], F32R, tag="z")
            znT = big.tile([128, KT, R], GDT, tag="z")
            ag_in = [dram.tile([D, 512], GDT, name=f"ag_in{k}")
                     for k in range(2)]
            ag_out = [dram.tile([N_CORES * D, 512], GDT, name=f"ag_out{k}",
                                addr_space="Local" if no_collective else "Shared")
                      for k in range(2)]
            rn_dram = dram.tile([R], F32)
            rs_in = [dram.tile([N_CORES, 512], F32, name=f"rs_in{k}")
                     for k in range(2)]
            rs_out = [dram.tile([512], F32, name=f"rs_out{k}")
                      for k in range(2)]

            # ---- phase 1, pipelined over row-halves jh ----
            with tc.tile_pool(name="xrow", bufs=2) as xrowp, \
                 tc.tile_pool(name="xr16", bufs=2) as xr16p, \
                 tc.tile_pool(name="xTp", bufs=2) as xTp, \
                 tc.tile_pool(name="sqp", bufs=2) as sqp, \
                 tc.tile_pool(name="ps1", bufs=2, space="PSUM") as ps1:
                for jh in range(2):
                    # relu+cast rows to bf16, then PE-transpose (bf16 1cyc/row)
                    xT = xTp.tile([128, 2 * KT, 512], BF16, tag="xT",
                                  name=f"xT{jh}")
                    for r in range(4):
                        rg = jh * 4 + r
                        xrow = xrowp.tile([128, 2 * E], F32, tag="xrow")
                        nc.sync.dma_start(xrow[:],
                                          x_in[rg * 128:(rg + 1) * 128, :])
                        xrow16 = xr16p.tile([128, 2 * E], BF16, tag="xr16")
                        nc.vector.tensor_scalar_max(xrow16[:], xrow[:], 0.0)
                        for cg in range(2 * E // 512):
                            pt = ps1.tile([128, 4, 128], BF16, tag="tp",
                                          bufs=3)
                            for q in range(4):
                                ct = cg * 4 + q
                                nc.tensor.transpose(
                                    pt[:, q, :],
                                    xrow16[:, ct * 128:(ct + 1) * 128],
                                    ident_b[:])
                            nc.vector.tensor_copy(
                                xT[:, cg * 4:(cg + 1) * 4,
                                   r * 128:(r + 1) * 128],
                                pt[:])
                    # project this half, interleaving the row-norm
                    # accumulation so pn completes right after the last tile
                    pn = ps1.tile([1, 512], F32, tag="pn", bufs=2,
                                  name=f"pn{jh}")
                    for h in range(2):
                        for m4 in range(P // 128):
                            ktile = h * 4 + m4
                            pz = ps1.tile([128, 512], F32, tag="pz", bufs=2)
                            for kt in range(E // 128):
                                nc.tensor.matmul(
                                    pz[:],
                                    w_r[:, kt, m4 * 128:(m4 + 1) * 128],
                                    xT[:, h * KT + kt, :],
                                    start=(kt == 0), stop=(kt == E // 128 - 1))
                            zsl = zT[:, ktile, jh * 512:(jh + 1) * 512]
                            nc.vector.tensor_scalar(
                                zsl, pz[:], b2[:, m4:m4 + 1], None, ALU.add)
                            sq = sqp.tile([128, 512], BF16, tag="sq")
                            nc.vector.tensor_tensor(sq[:], zsl, zsl, ALU.mult)
                            nc.tensor.matmul(pn[:], ones_b[:], sq[:],
                                             start=(ktile == 0),
                                             stop=(ktile == KT - 1))
                    nrm = small.tile([1, 512], F32, tag="nrm", name=f"nrm{jh}")
                    # sqrt(pn)/SCL, so the reciprocal below yields SCL/||z||
                    nc.scalar.activation(nrm[:], pn[:], AF.Sqrt,
                                         scale=1.0 / (SCL * SCL))
                    nc.scalar.dma_start(rn_dram[None, jh * 512:(jh + 1) * 512],
                                        nrm[:])
                    nrm_bc = small.tile([128, 512], F32, tag="nrmbc",
                                        name=f"nrmbc{jh}")
                    nc.scalar.dma_start(
                        nrm_bc[:],
                        rn_dram[None, jh * 512:(jh + 1) * 512]
                        .to_broadcast([128, 512]))
                    # reciprocal on all 128 partitions (1-lane recip is slow)
                    nc.vector.tensor_scalar_max(nrm_bc[:], nrm_bc[:], EPS)
                    nc.vector.reciprocal(
                        rn_bc[:, jh * 512:(jh + 1) * 512], nrm_bc[:])
                    # normalize and ship this half
                    for kt in range(KT):
                        nc.vector.tensor_tensor(
                            znT[:, kt, jh * 512:(jh + 1) * 512],
                            zT[:, kt, jh * 512:(jh + 1) * 512],
                            rn_bc[:, jh * 512:(jh + 1) * 512], ALU.mult)
                    nc.scalar.dma_start(
                        ag_in[jh].rearrange("(kt p) j -> p kt j", p=128),
                        znT[:, :, jh * 512:(jh + 1) * 512])
                    if no_collective:
                        for c in range(N_CORES):
                            nc.sync.dma_start(
                                ag_out[jh][c * D:(c + 1) * D, :], ag_in[jh][:])
                    else:
                        nc.gpsimd.collective_compute(
                            "AllGather", ALU.bypass,
                            replica_groups=[list(range(N_CORES))],
                            ins=[ag_in[jh][:]], outs=[ag_out[jh][:]])

            # ---- phase 2: symmetric blockwise cos-sim ----
            rowsum = const.tile([128, MT, NSLOT], F32)
            pos_acc = const.tile([128, MT], F32)
            corr_acc = const.tile([128, MT], F32)

            pid = nc.sync.partition_id()

            def gemm_tile(pg, rhs_ap, m):
                # fp8 DoubleRow: each matmul contracts two 128-feature blocks
                for t in range(KT // 2):
                    nc.tensor.matmul(pg[:],
                                     znT[:, 2 * t:2 * t + 2,
                                         m * 128:(m + 1) * 128],
                                     rhs_ap[:, 2 * t:2 * t + 2, :],
                                     start=(t == 0), stop=(t == KT // 2 - 1),
                                     perf_mode=DR)

            with tc.tile_pool(name="rhs", bufs=2) as rhsp, \
                 tc.tile_pool(name="junk", bufs=3) as junkp, \
                 tc.tile_pool(name="dtmp", bufs=4) as dtmpp, \
                 tc.tile_pool(name="ps2", bufs=1, space="PSUM") as ps2:

                # zero the ReduceScatter inputs (unwritten slots must be 0)
                zb = small.tile([N_CORES, 512], F32)
                nc.vector.memset(zb[:], 0.0)
                for k in range(2):
                    nc.sync.dma_start(rs_in[k][:], zb[:])

                # local block d=0 first — overlaps the collectives
                for nb in range(2):
                    for m in range(MT):
                        pg = ps2.tile([128, 512], F32, tag="pg", bufs=5,
                                      name=f"pgl{nb}_{m}")
                        gemm_tile(pg, znT[:, :, nb * 512:(nb + 1) * 512], m)
                        junk = junkp.tile([128, 512], BF16, tag="junk")
                        nc.scalar.activation(
                            junk[:], pg[:], AF.Exp, scale=INV_TS,
                            accum_out=rowsum[:, m, nb:nb + 1])
                        if nb == m // 4:
                            # self-similarity at compile-time position
                            off = (m % 4) * 128
                            jd = junkp.tile([128, 128], F32, tag="jd")
                            nc.vector.tensor_tensor(
                                jd[:], pg[:, off:off + 128], ident[:],
                                ALU.mult)
                            d = dtmpp.tile([128, 1], F32, tag="d")
                            nc.vector.reduce_sum(d[:], jd[:],
                                                 axis=mybir.AxisListType.X)
                            nc.scalar.activation(
                                corr_acc[:, m:m + 1], d[:], AF.Exp,
                                scale=INV_TS)

                # remote blocks d = 1..4, per gathered chunk k
                for k in range(2):
                    for dd in range(1, 5):
                        slot = 2 + (dd - 1) * 2 + k
                        row0 = ((pid + dd) % N_CORES) * D
                        rhs = rhsp.tile([128, KT, 512], GDT, tag="rhs")
                        src = ag_out[k][bass.ds(row0, D), :].rearrange(
                            "(kt p) j -> p kt j", p=128)
                        nc.sync.dma_start(rhs[:], src)
                        cs = None
                        if dd < 4:
                            cs = ps2.tile([1, 512], F32, tag="cs", bufs=2,
                                          name=f"cs{k}_{dd}")
                        for m in range(MT):
                            pg = ps2.tile([128, 512], F32, tag="pg", bufs=5,
                                          name=f"pg{k}_{dd}_{m}")
                            gemm_tile(pg, rhs, m)
                            junk = junkp.tile([128, 512], BF16, tag="junk")
                            nc.scalar.activation(
                                junk[:], pg[:], AF.Exp, scale=INV_TS,
                                accum_out=rowsum[:, m, slot:slot + 1])
                            if dd < 4:
                                nc.tensor.matmul(cs[:], ones_b[:], junk[:],
                                                 start=(m == 0),
                                                 stop=(m == MT - 1))
                            if dd == 4 and k == m // 4:
                                # positive-pair logit on the block diagonal
                                off = (m % 4) * 128
                                jd = junkp.tile([128, 128], F32, tag="jd")
                                nc.vector.tensor_tensor(
                                    jd[:], pg[:, off:off + 128], ident[:],
                                    ALU.mult)
                                dpos = dtmpp.tile([128, 1], F32, tag="dp")
                                nc.vector.reduce_sum(
                                    dpos[:], jd[:], axis=mybir.AxisListType.X)
                                nc.vector.tensor_scalar_mul(
                                    pos_acc[:, m:m + 1], dpos[:], INV_TS)
                        if dd < 4:
                            # ship this block's column sums to core (c+dd)
                            css = dtmpp.tile([1, 512], F32, tag="css",
                                             name=f"css{k}_{dd}")
                            nc.vector.tensor_copy(css[:], cs[:])
                            nc.sync.dma_start(
                                rs_in[k][bass.ds((pid + dd) % N_CORES, 1), :],
                                css[:])
                    # per-chunk ReduceScatter: k=0's overlaps k=1 compute
                    if dd == 4:
                        if no_collective:
                            nc.sync.dma_start(rs_out[k][None, :],
                                              rs_in[k][0:1, :])
                        else:
                            nc.gpsimd.collective_compute(
                                "ReduceScatter", ALU.add,
                                replica_groups=[list(range(N_CORES))],
                                ins=[rs_in[k][:]], outs=[rs_out[k][:]])

                # ---- finale: lse, nll, partial sum (batched over m) ----
                rs = small.tile([128, MT], F32)
                nc.vector.reduce_sum(rs[:], rowsum[:],
                                     axis=mybir.AxisListType.X)
                rcv = small.tile([128, MT], F32)
                for k in range(2):
                    nc.sync.dma_start(
                        rcv[:, k * 4:(k + 1) * 4],
                        rs_out[k].rearrange("(m p) -> p m", p=128))
                nc.vector.tensor_tensor(rs[:], rs[:], rcv[:], ALU.add)
                nc.vector.tensor_tensor(rs[:], rs[:], corr_acc[:], ALU.subtract)
                lse = small.tile([128, MT], F32)
                nc.scalar.activation(lse[:], rs[:], AF.Ln)
                nll = small.tile([128, MT], F32)
                nc.vector.tensor_tensor(nll[:], lse[:], pos_acc[:], ALU.subtract)
                pf = ps2.tile([1, MT], F32, tag="pf", bufs=1)
                nc.tensor.matmul(pf[:], ones_f[:], nll[:], start=True, stop=True)
                fs = small.tile([1, MT], F32)
                nc.vector.tensor_copy(fs[:], pf[:])
                nc.sync.dma_start(out[:], fs[:])

    nc.compile()
    return nc


def kernel(x, w, b):
    if "nc" not in _cached:
        _cached["nc"] = _build()
    nc = _cached["nc"]
    x = np.ascontiguousarray(np.asarray(x, dtype=np.float32))
    w = np.ascontiguousarray(np.asarray(w, dtype=np.float32))
    b = np.ascontiguousarray(np.asarray(b, dtype=np.float32))
    in_maps = [{
        "xs": np.ascontiguousarray(x[c * R:(c + 1) * R]),
        "w": w, "b": b,
    } for c in range(N_CORES)]
    res = run_bass_kernel_spmd(nc, in_maps, list(range(N_CORES)))
    total = 0.0
    for c in range(N_CORES):
        total += float(res.results[c]["out"].astype(np.float64).sum())
    return np.float32(total / B)

